# revision 41
# baseline (speedup 1.0000x reference)
"""Trainium2 Bass kernel for CoocOpModel.

out[b,s,z] = sum_{i,j} func[b,s,i] * cooc[i,j,z] * arg[b,s,j]
  with func = func_and_arg[..., :128], arg = func_and_arg[..., 128:]

Shapes (hardcoded): func_and_arg [4,1024,256] f32, cooccurrences [128,128,128] f32,
out [4,1024,128] f32.  D = 128, tokens T = 4096.

Strategy: data-parallel over tokens across 8 cores (512 tokens/core).

Per-core math (t = local token index in [0,512)):
  Lane packing: each SBUF partition (matmul contraction lane) is a pair
  lane = (i_sub, j_sub), i_sub in [0,8), j_sub in [0,16).
  Matmul m = (b, c), b in [0,16), c in [0,8) contracts 128 (i,j) pairs:
    i = 8*b + i_sub,  j = 16*c + j_sub
    out[z, t] += sum_lane  c3[lane, m, z] * G_m[lane, t]
    c3[lane, (m, z)] = cooc[8b+i_sub, 16c+j_sub, z]     (host-rearranged)
    G_m[lane, t]     = f[8b+i_sub, t] * a[16c+j_sub, t] (DVE tensor_tensor)
  via replicated operands in SBUF:
    f_rep[lane, (b, t)] = f[8b+i_sub, t]   (2 MB, 16x replication over j_sub)
    a_rep[lane, (c, t)] = a[16c+j_sub, t]  (1 MB,  8x replication over i_sub)
  so total DMA is ~7.25 MB/core instead of 20.4 MB/core for the naive
  1-i-per-matmul layout (which needs f broadcast to all 128 partitions).

Bottleneck model: DVE tensor_tensor builds G (65536 f16 elems/lane at 2x
mode ~= 34 us busy) and paces the PE (128 matmuls, ~216 ns warm each,
~28 us).  DMA is ~7.25 MB/core and hides under the DVE span, but the
early delivery ramp (~0.2 MB/us/queue from ~7.5 us after the framework
preamble) gates the pipeline fill, so:
  - the TT ladder runs in two c-phases (c=0..3 then c=4..7), b-major,
    so first-touch bytes per chunk are small and a[c4-7] plus half of
    c3 are not needed until the kernel midpoint;
  - chunk granularity is fine where data-starved (head singles) and
    coarse where data-rich (b-pair doubles, fewer per-TT overheads);
  - all DMA dispatches are issued upfront on the two HWDGE queues as
    few, need-ordered pieces (each dispatch costs ~0.65 us of queue
    time; each piece sized to land just before its first consumer);
  - c3 is host-packed in stationary consumption order; the tail chunk
    is split so only 2 matmuls drain after the last TT;
  - ~26 dummy matmuls on a zeroed tile warm the PE (HAM K=8/8) during
    the DMA fill, so no real matmul runs clock-gated;
  - the accumulation is phase-split across two PSUM banks (psA final at
    the kernel midpoint, ACT-copied to SBUF under phase B), so the final
    evacuation is two DVE adds (SBUF + PSUM -> f16) with their output
    DMA dispatches interleaved — no whole-tile false deps against the
    trailing matmuls (note: DVE can read at most one PSUM operand).
Measured: 84.3 us (prior session baseline) -> ~55.3-56.9 us.  Runs on
a power-throttled chip (all-engine clocks ~1.2x slower, PE pinned at
HAM K=4/8) measure ~66-68 us regardless of kernel version.
"""

import sys

sys.path.insert(0, "/opt/trn_rl_repo")

import numpy as np
from contextlib import ExitStack

import concourse.bass as bass
import concourse.tile as tile
from concourse import bacc, mybir
from concourse.bass_utils import run_bass_kernel_spmd

F16 = mybir.dt.float16
F32 = mybir.dt.float32
NP_F16 = np.float16

N_CORES = 8
D = 128
T_TOTAL = 4096
T_CORE = T_TOTAL // N_CORES  # 512

P_I = 8    # i_sub values per lane group
P_J = 16   # j_sub values
NB = 16    # b blocks: i = 8b + i_sub
NCC = 8    # c blocks: j = 16c + j_sub

_NC_CACHE = None


def _build():
    nc = bacc.Bacc("TRN2", target_bir_lowering=False, debug=False, num_devices=N_CORES)

    # host-replicated operands (see _prep_in_maps):
    #   f_in[lane, b*512+t] = f[8b+i_sub, t],  a_in[lane, c*512+t] = a[16c+j_sub, t]
    f_in = nc.dram_tensor("f_rep", [D, NB * T_CORE], F16, kind="ExternalInput").ap()
    a_in = nc.dram_tensor("a_rep", [D, NCC * T_CORE], F16, kind="ExternalInput").ap()
    # c3[lane, k*128 + z]: stationaries packed in matmul emission order
    # (phase-major, b, ci) — see _prep_in_maps
    c3 = nc.dram_tensor("c3", [D, D * D], F16, kind="ExternalInput").ap()
    out_t = nc.dram_tensor("out_t", [D, T_CORE], F16, kind="ExternalOutput").ap()

    with tile.TileContext(nc) as tc:
        with ExitStack() as ctx:
            const_pool = ctx.enter_context(tc.tile_pool(name="const", bufs=1))
            g_pool = ctx.enter_context(tc.tile_pool(name="g", bufs=6))
            out_pool = ctx.enter_context(tc.tile_pool(name="out", bufs=1))
            psum_pool = ctx.enter_context(
                tc.tile_pool(name="psum", bufs=1, space="PSUM")
            )

            a_rep = const_pool.tile([D, NCC * T_CORE], F16, tag="arep")  # [lane,(c,t)]
            f_rep = const_pool.tile([D, NB * T_CORE], F16, tag="frep")   # [lane,(b,t)]
            c_sb = const_pool.tile([D, D * D], F16, tag="c3")            # [lane,(m,z)]

            # ---- all DMA dispatches upfront ------------------------------
            # q0 = sync, q1 = scalar (two HWDGE queues, FIFO each), pieces
            # in need order, each sized to land just before its first
            # consumer (a dispatch occupies the issuing engine ~0.65 us, so
            # few, mid-size pieces beat many small ones).
            q0, q1 = nc.sync, nc.scalar

            q0.dma_start(a_rep[:, 0 : 2 * T_CORE], a_in[:, 0 : 2 * T_CORE])
            q0.dma_start(a_rep[:, 2 * T_CORE : 4 * T_CORE], a_in[:, 2 * T_CORE : 4 * T_CORE])
            q0.dma_start(f_rep[:, 3 * T_CORE : 5 * T_CORE], f_in[:, 3 * T_CORE : 5 * T_CORE])
            q0.dma_start(f_rep[:, 5 * T_CORE : 7 * T_CORE], f_in[:, 5 * T_CORE : 7 * T_CORE])
            q0.dma_start(f_rep[:, 7 * T_CORE : 10 * T_CORE], f_in[:, 7 * T_CORE : 10 * T_CORE])
            q0.dma_start(f_rep[:, 10 * T_CORE :], f_in[:, 10 * T_CORE :])
            q0.dma_start(a_rep[:, 4 * T_CORE :], a_in[:, 4 * T_CORE :])

            q1.dma_start(c_sb[:, 0 : 8 * D], c3[:, 0 : 8 * D])
            q1.dma_start(f_rep[:, 0:T_CORE], f_in[:, 0:T_CORE])
            q1.dma_start(f_rep[:, T_CORE : 3 * T_CORE], f_in[:, T_CORE : 3 * T_CORE])
            q1.dma_start(c_sb[:, 8 * D : 16 * D], c3[:, 8 * D : 16 * D])
            q1.dma_start(c_sb[:, 16 * D : 32 * D], c3[:, 16 * D : 32 * D])
            q1.dma_start(c_sb[:, 32 * D : 64 * D], c3[:, 32 * D : 64 * D])
            q1.dma_start(c_sb[:, 64 * D : 96 * D], c3[:, 64 * D : 96 * D])
            q1.dma_start(c_sb[:, 96 * D :], c3[:, 96 * D :])

            # ---- compute: TT chunk -> matmuls, one PSUM accumulator.
            # Chunk list entries: (b, c0, nc_, nb) — TT covers b..b+nb-1
            # (nb>1 in data-rich regions) x c0..c0+nc_-1.
            # Stationary index advances in emission order; c3 host layout
            # matches (phase-major, b, ci).
            chunks = [(0, 0, 2, 1), (0, 2, 2, 1)]
            chunks += [(b, 0, 4, 1) for b in range(1, 8)]          # phase A singles
            chunks += [(b, 0, 4, 2) for b in range(8, NB, 2)]      # phase A doubles
            chunks += [(b, 4, 4, 2) for b in range(0, NB - 2, 2)]  # phase B doubles
            chunks += [(NB - 2, 4, 4, 1)]                          # tail single
            chunks += [(NB - 1, 4, 2, 1), (NB - 1, 6, 2, 1)]       # tail halves

            # PE warm-up: HAM starts the PE clock-gated at K=4/8 and only
            # un-throttles after ~3.4us of sustained activity.  Real
            # matmuls can't start until c3/G arrive (~13us), so run dummy
            # matmuls on a zeroed tile into a scratch PSUM bank from ~7.6us
            # — the first real matmul then issues already warm, avoiding a
            # ~427ns-per-MM cold spacing backlog that otherwise persists to
            # the kernel tail.
            z_tile = const_pool.tile([D, T_CORE], F16, tag="warmz")
            nc.gpsimd.memset(z_tile[:], 0.0)
            ps_warm = psum_pool.tile([D, T_CORE], F32, tag="warm")
            for _ in range(26):
                nc.tensor.matmul(
                    ps_warm[:], z_tile[:, 0:D], z_tile[:], start=True, stop=True
                )

            # Two accumulators: phase A (m 0..63) and phase B (m 64..127) in
            # separate PSUM banks.  out = psA + psB, computed as two DVE
            # tensor_tensor adds (f32 PSUM -> f16 SBUF) at the end — this
            # keeps the final evacuation free of whole-tile false deps
            # against the trailing matmuls, and fuses add + downcast.
            psA = psum_pool.tile([D, T_CORE], F32, tag="psA")
            psB = psum_pool.tile([D, T_CORE], F32, tag="psB")
            f_ap = f_rep[:]
            m_idx = 0
            M_TOT = D
            M_HALF = D // 2
            for b, c0, nc_, nb in chunks:
                gt = g_pool.tile([D, 8 * T_CORE], F16, tag="g")
                ncol = nb * nc_ * T_CORE
                # G[lane, (b', c, t)] = a_rep[lane, (c, t)] * f_rep[lane, (b', t)]
                f_view = bass.AP(
                    f_ap.tensor,
                    f_ap.offset + b * T_CORE,
                    ([f_ap.ap[0], [T_CORE, nb], [0, nc_], [1, T_CORE]]
                     if nb > 1
                     else [f_ap.ap[0], [0, nc_], [1, T_CORE]]),
                )
                a_view = bass.AP(
                    a_rep[:].tensor,
                    a_rep[:].offset + c0 * T_CORE,
                    ([a_rep[:].ap[0], [0, nb], [1, nc_ * T_CORE]]
                     if nb > 1
                     else [a_rep[:].ap[0], [1, nc_ * T_CORE]]),
                )
                nc.vector.tensor_mul(gt[:, 0:ncol], a_view, f_view)
                for u in range(nb * nc_):
                    ps = psA if m_idx < M_HALF else psB
                    nc.tensor.matmul(
                        ps[:],
                        c_sb[:, m_idx * D : (m_idx + 1) * D],
                        gt[:, u * T_CORE : (u + 1) * T_CORE],
                        start=(m_idx == 0 or m_idx == M_HALF),
                        stop=(m_idx == M_HALF - 1 or m_idx == M_TOT - 1),
                    )
                    m_idx += 1
            assert m_idx == M_TOT

            # psA is final at the kernel midpoint; ACT (idle then) copies it
            # to SBUF, fully hidden under phase B.  (DVE can read at most
            # one PSUM operand per instruction, so the final add needs one
            # side in SBUF.)
            sA = out_pool.tile([D, T_CORE], F32, tag="sA")
            nc.scalar.copy(sA[:], psA[:])

            # final evacuation: out = sA + psB, two DVE adds (lo half, then
            # hi half), each followed immediately by its output DMA dispatch
            # so the first transfer overlaps the second add.
            h = T_CORE // 2
            o_sb = out_pool.tile([D, T_CORE], F16, tag="o")
            nc.vector.tensor_add(o_sb[:, 0:h], sA[:, 0:h], psB[:, 0:h])
            q0.dma_start(out_t[:, 0:h], o_sb[:, 0:h])
            nc.vector.tensor_add(o_sb[:, h:], sA[:, h:], psB[:, h:])
            q1.dma_start(out_t[:, h:], o_sb[:, h:])

    nc.compile()
    return nc


def _get_nc():
    global _NC_CACHE
    if _NC_CACHE is None:
        _NC_CACHE = _build()
    return _NC_CACHE


def _prep_in_maps(func_and_arg, cooccurrences):
    fa = np.asarray(func_and_arg, dtype=np.float32).reshape(T_TOTAL, 2 * D)
    c = np.asarray(cooccurrences, dtype=np.float32)
    # c3[(i_sub, j_sub), (phase, b, ci, z)] = cooc[8b+i_sub, 16*(4*phase+ci)+j_sub, z]
    # (consumption order of the two-c-phase, b-major chunk ladder)
    c3 = np.ascontiguousarray(
        c.reshape(NB, P_I, 2, 4, P_J, D)
        .transpose(1, 4, 2, 0, 3, 5)
        .reshape(D, D * D)
    ).astype(NP_F16)
    in_maps = []
    for core in range(N_CORES):
        s = fa[core * T_CORE : (core + 1) * T_CORE]  # [512, 256]
        f_tc = s[:, :D].T.astype(NP_F16)  # [128 i, 512 t]
        a_tc = s[:, D:].T.astype(NP_F16)  # [128 j, 512 t]
        # f_rep[(i_sub, j_sub), (b, t)] = f[8b+i_sub, t]
        f_rep = np.ascontiguousarray(
            np.broadcast_to(
                f_tc.reshape(NB, P_I, T_CORE).transpose(1, 0, 2)[:, None, :, :],
                (P_I, P_J, NB, T_CORE),
            )
        ).reshape(D, NB * T_CORE)
        # a_rep[(i_sub, j_sub), (c, t)] = a[16c+j_sub, t]
        a_rep = np.ascontiguousarray(
            np.broadcast_to(
                a_tc.reshape(NCC, P_J, T_CORE).transpose(1, 0, 2)[None, :, :, :],
                (P_I, P_J, NCC, T_CORE),
            )
        ).reshape(D, NCC * T_CORE)
        in_maps.append({"f_rep": f_rep, "a_rep": a_rep, "c3": c3})
    return in_maps


def kernel(func_and_arg: np.ndarray, cooccurrences: np.ndarray) -> np.ndarray:
    assert func_and_arg.shape == (4, 1024, 2 * D)
    assert cooccurrences.shape == (D, D, D)

    in_maps = _prep_in_maps(func_and_arg, cooccurrences)
    nc = _get_nc()
    res = run_bass_kernel_spmd(nc, in_maps, core_ids=list(range(N_CORES)))

    # out_t per core: [z=128, t=512] f16 -> [t, z]; concat -> [4096, 128]
    outs = [res.results[c]["out_t"].astype(np.float32).T for c in range(N_CORES)]
    out = np.concatenate(outs, axis=0).reshape(4, 1024, D)
    return np.ascontiguousarray(out)
